# revision 6
# baseline (speedup 1.0000x reference)
"""Trainium2 Bass kernel for the sparse-attention (local 3x3 unfold) problem.

Math (per batch-channel (b,c), H=W=128, K=3, pad=1):
  ku = unfold(key)  -> [9, L] raw-flat, reinterpreted [L, 9]
  qu = unfold(query)
  out1 = ku * qu[:, 4:5] ; out2 = ku[:, 4:5] * qu   (as [L, 9] views)

Key observation: the raw flat unfold stream m = 0..9L-1 is patch-major —
it is literally nine [128,128] windows of the zero-padded image
concatenated.  The host therefore materializes the unfold with nine
contiguous array slices (no gather), pre-casts to fp16, and lays it out
in exact SBUF tile order [group, partition, ch, 1152].  On device each
8-channel group is then:
  2 loads + 2 multiplies + 2 stores,
every DMA a [128 x 18.4 KiB-descriptor] contiguous transfer (measured
~24 B/ns per DMA engine vs ~9 B/ns for the 2.3 KiB descriptors forced
by on-device unfold assembly).  The multiply is the stride-9 center
broadcast: out[r, 9g+e] = Tk[r, 9g+e] * Tq[r, 9g+4], one DVE instr per
output per group.  Outputs stay in the same device layout (fp16) and the
host permutes/upcasts.  Device traffic is the 2x9L fp16 operand streams
in and the 2x9L fp16 products out: ~37.8 MB/core, DMA-engine-bound.

Sharding: pure data-parallel over the 256 (b,c) channels; 32 per core.
"""

import sys

for _p in ("/opt/trn_rl_repo", "/opt/pypackages"):
    if _p not in sys.path:
        sys.path.insert(0, _p)

import numpy as np

import concourse.bass as bass
import concourse.mybir as mybir
import concourse.tile as tile
from concourse.bass import AP
from concourse.bass_utils import run_bass_kernel_spmd
from concourse.vector_clock import ScopedClock

# ---------------------------------------------------------------------------
# Patch: this container's walrus rejects >1 sync-wait on the Tile tail Drain
# ("Too many sync wait commands").  Spill extra waits onto SP NOPs, which
# execute in program order before the all-engine barrier, preserving the
# "all work done before sem clear" semantics.
# ---------------------------------------------------------------------------


def _drain_and_barrier(self, tick_clock, wait_clock):
    nc = self.nc
    drain_inst = nc.sync.drain()
    wait_clock.add_sem_waits(
        drain_inst.ins, ScopedClock({None: tick_clock.global_clock})
    )
    si = drain_inst.ins.sync_info
    if si is not None and len(si.on_wait) > 1:
        waits = list(si.on_wait)
        drain_inst.ins.sync_info = mybir.SyncInfo(
            on_wait=waits[:1], on_update=list(si.on_update)
        )
        for w in waits[1:]:
            nop = nc.sync.nop(nofuse=True)
            nop.ins.sync_info = mybir.SyncInfo(on_wait=[w], on_update=[])

    nc.all_engine_barrier()
    assert self.sems is not None
    popped = nc._tile_sem_poison_stack.pop()
    assert popped is self._sem_poison
    nc.clear_and_free_semaphores(list(self.sems.allocated().values()))
    nc.all_engine_barrier()


tile.TileContext._drain_and_barrier = _drain_and_barrier


def _split_waits(nc, maxw=1):
    """Walrus here allows only `maxw` sync-waits per instruction: move extra
    waits onto same-engine NOPs inserted immediately before the instruction
    (same engine stream => executes before it)."""
    for fn in nc.m.functions:
        for bb in fn.blocks:
            out = []
            for inst in bb.instructions:
                si = getattr(inst, "sync_info", None)
                if si is not None and len(si.on_wait) > maxw:
                    waits = list(si.on_wait)
                    for w in waits[:-maxw]:
                        nop = mybir.InstNoOp(
                            name=nc.get_next_instruction_name(),
                            bass_nofuse=True,
                        )
                        nop.engine = inst.engine
                        nop.sync_info = mybir.SyncInfo(on_wait=[w], on_update=[])
                        nc.register_instruction(nop)
                        out.append(nop)
                    inst.sync_info = mybir.SyncInfo(
                        on_wait=waits[-maxw:], on_update=list(si.on_update)
                    )
                out.append(inst)
            bb.instructions[:] = out

# ---------------------------------------------------------------------------

F16 = mybir.dt.float16

N_CORES = 8
B, C, H, W = 4, 64, 128, 128
BC = B * C                # 256 channels
CPC = BC // N_CORES       # 32 channels per core
NCH = 8                   # channels per group (one tile set)
NG = CPC // NCH           # groups per core
L = H * W
CH_FREE = 9 * 128         # 1152 elems per channel per partition
FREE = NCH * CH_FREE      # tile free width (9216)
OUT_CH = 9 * L            # 147456 elems per channel
G_ELEM = 128 * FREE       # elems per group buffer [128, FREE]


def _build_program():
    nc = bass.Bass(trn_type="TRN2")
    # host-prepared unfold operands in exact tile layout
    # [NG, 128, NCH, 1152] per input
    uk = nc.dram_tensor("uk", [NG * G_ELEM], F16, kind="ExternalInput")
    uq = nc.dram_tensor("uq", [NG * G_ELEM], F16, kind="ExternalInput")
    # outputs in the same layout (host permutes back)
    o1 = nc.dram_tensor("o1", [NG * G_ELEM], F16, kind="ExternalOutput")
    o2 = nc.dram_tensor("o2", [NG * G_ELEM], F16, kind="ExternalOutput")

    engines = [nc.sync, nc.scalar]
    eng_i = [0]

    def eng():
        e = engines[eng_i[0] % len(engines)]
        eng_i[0] += 1
        return e

    HF = FREE // 2  # free-dim half (4608): finer pipeline granularity

    def half(base, h):
        return [[FREE, 128], [1, HF]], base + h * HF

    def mul_aps(o, a, b, h):
        off = h * HF
        lin = [[FREE, 128], [9, HF // 9], [1, 9]]
        bc = [[FREE, 128], [9, HF // 9], [0, 9]]
        return (AP(o, off, lin), AP(a, off, lin), AP(b, off + 4, bc))

    # multiply halves: 16 total; DVE @ ~4.8us/half, gpsimd (Multiply
    # impl eff 0.42 @1.2GHz) ~9.2us/half -> give gpsimd 6, DVE 10
    mult_i = [0]

    def mul_eng():
        i = mult_i[0]
        mult_i[0] += 1
        return nc.gpsimd if i % 8 in (1, 3, 5) else nc.vector

    with tile.TileContext(nc) as tc:
        with (
            tc.tile_pool(name="tin", bufs=2) as tin,
            tc.tile_pool(name="tout", bufs=2) as tout,
        ):
            for g in range(NG):
                tk = tin.tile([128, FREE], F16, tag="tk")
                tq = tin.tile([128, FREE], F16, tag="tq")
                o1t = tout.tile([128, FREE], F16, tag="o1t")
                o2t = tout.tile([128, FREE], F16, tag="o2t")
                tkh, tqh = tk[:].tensor, tq[:].tensor
                o1h, o2h = o1t[:].tensor, o2t[:].tensor
                for h in (0, 1):
                    for src_d, th in ((uk, tkh), (uq, tqh)):
                        ap, off = half(g * G_ELEM, h)
                        eng().dma_start(AP(th, h * HF, ap),
                                        AP(src_d, off, ap))
                    mul_eng().tensor_mul(*mul_aps(o1h, tkh, tqh, h))
                    mul_eng().tensor_mul(*mul_aps(o2h, tqh, tkh, h))
                    for od, oth in ((o1, o1h), (o2, o2h)):
                        ap, off = half(g * G_ELEM, h)
                        eng().dma_start(AP(od, off, ap),
                                        AP(oth, h * HF, ap))
    _split_waits(nc)
    return nc


_NC_CACHE = []


def _get_nc():
    if not _NC_CACHE:
        _NC_CACHE.append(_build_program())
    return _NC_CACHE[0]


def _unfold_dev(x):
    """[B,C,H,W] f32 -> [BC, 9L] fp16 raw-flat unfold (nine contiguous
    padded-image windows), then regrouped to device tile order
    [BC/NCH groups of (128, NCH, 1152)] per core slice later."""
    xpad = np.pad(
        np.ascontiguousarray(x, dtype=np.float32).reshape(BC, H, W),
        ((0, 0), (1, 1), (1, 1)),
    ).astype(np.float16)
    u = np.empty((BC, 9, L), np.float16)
    for p in range(9):
        di, dj = divmod(p, 3)
        u[:, p, :] = xpad[:, di : di + H, dj : dj + W].reshape(BC, L)
    # [BC, 9L] -> [BC, 128, 1152] (m = 1152 r + f) -> core/group layout
    return u.reshape(BC, 128, CH_FREE)


def make_in_maps(key_map, query_map):
    ku = _unfold_dev(key_map)
    qu = _unfold_dev(query_map)
    maps = []
    for m in range(N_CORES):
        sl = slice(m * CPC, (m + 1) * CPC)
        # [CPC, 128, 1152] -> [NG, NCH, 128, 1152] -> [NG, 128, NCH, 1152]
        def dev(u):
            return np.ascontiguousarray(
                u[sl].reshape(NG, NCH, 128, CH_FREE).transpose(0, 2, 1, 3)
            ).reshape(-1)

        maps.append({"uk": dev(ku), "uq": dev(qu)})
    return maps


def assemble(results):
    # device layout [NG, 128, NCH, 1152] -> per-channel [CPC, 147456]
    def unshuffle(o):
        return (
            o.reshape(NG, 128, NCH, CH_FREE)
            .transpose(0, 2, 1, 3)
            .reshape(CPC, OUT_CH)
        )

    out1 = np.concatenate(
        [unshuffle(results[m]["o1"]) for m in range(N_CORES)], axis=0
    )
    out2 = np.concatenate(
        [unshuffle(results[m]["o2"]) for m in range(N_CORES)], axis=0
    )
    return (
        out1.astype(np.float32).reshape(B, C, L, 9),
        out2.astype(np.float32).reshape(B, C, L, 9),
    )


def kernel(key_map, query_map):
    nc = _get_nc()
    in_maps = make_in_maps(key_map, query_map)
    res = run_bass_kernel_spmd(nc, in_maps, core_ids=list(range(N_CORES)))
    return assemble(res.results)


# revision 7
# speedup vs baseline: 1.3896x; 1.3896x over previous
"""Trainium2 Bass kernel for the sparse-attention (local 3x3 unfold) problem.

Math (per batch-channel (b,c), H=W=128, K=3, pad=1):
  ku = unfold(key)  -> [9, L] raw-flat, reinterpreted [L, 9]
  qu = unfold(query)
  out1 = ku * qu[:, 4:5] ; out2 = ku[:, 4:5] * qu   (as [L, 9] views)

Key observation: the raw flat unfold stream m = 0..9L-1 is patch-major —
it is literally nine [128,128] windows of the zero-padded image
concatenated.  The host therefore materializes the unfold with nine
contiguous array slices (no gather), pre-casts to fp16, and lays it out
in exact SBUF tile order [group, partition, ch, 1152].  On device each
8-channel group is then:
  2 loads + 2 multiplies + 2 stores,
every DMA a [128 x 18.4 KiB-descriptor] contiguous transfer (measured
~24 B/ns per DMA engine vs ~9 B/ns for the 2.3 KiB descriptors forced
by on-device unfold assembly).  The multiply is the stride-9 center
broadcast: out[r, 9g+e] = Tk[r, 9g+e] * Tq[r, 9g+4], one DVE instr per
output per group.  Outputs stay in the same device layout (fp16) and the
host permutes/upcasts.  Device traffic is the 2x9L fp16 operand streams
in and the 2x9L fp16 products out: ~37.8 MB/core, DMA-engine-bound.

Sharding: pure data-parallel over the 256 (b,c) channels; 32 per core.
"""

import sys

for _p in ("/opt/trn_rl_repo", "/opt/pypackages"):
    if _p not in sys.path:
        sys.path.insert(0, _p)

import numpy as np

import concourse.bass as bass
import concourse.mybir as mybir
import concourse.tile as tile
from concourse.bass import AP
from concourse.bass_utils import run_bass_kernel_spmd
from concourse.vector_clock import ScopedClock

# ---------------------------------------------------------------------------
# Patch: this container's walrus rejects >1 sync-wait on the Tile tail Drain
# ("Too many sync wait commands").  Spill extra waits onto SP NOPs, which
# execute in program order before the all-engine barrier, preserving the
# "all work done before sem clear" semantics.
# ---------------------------------------------------------------------------


def _drain_and_barrier(self, tick_clock, wait_clock):
    nc = self.nc
    drain_inst = nc.sync.drain()
    wait_clock.add_sem_waits(
        drain_inst.ins, ScopedClock({None: tick_clock.global_clock})
    )
    si = drain_inst.ins.sync_info
    if si is not None and len(si.on_wait) > 1:
        waits = list(si.on_wait)
        drain_inst.ins.sync_info = mybir.SyncInfo(
            on_wait=waits[:1], on_update=list(si.on_update)
        )
        for w in waits[1:]:
            nop = nc.sync.nop(nofuse=True)
            nop.ins.sync_info = mybir.SyncInfo(on_wait=[w], on_update=[])

    nc.all_engine_barrier()
    assert self.sems is not None
    popped = nc._tile_sem_poison_stack.pop()
    assert popped is self._sem_poison
    nc.clear_and_free_semaphores(list(self.sems.allocated().values()))
    nc.all_engine_barrier()


tile.TileContext._drain_and_barrier = _drain_and_barrier


def _split_waits(nc, maxw=1):
    """Walrus here allows only `maxw` sync-waits per instruction: move extra
    waits onto same-engine NOPs inserted immediately before the instruction
    (same engine stream => executes before it)."""
    for fn in nc.m.functions:
        for bb in fn.blocks:
            out = []
            for inst in bb.instructions:
                si = getattr(inst, "sync_info", None)
                if si is not None and len(si.on_wait) > maxw:
                    waits = list(si.on_wait)
                    for w in waits[:-maxw]:
                        nop = mybir.InstNoOp(
                            name=nc.get_next_instruction_name(),
                            bass_nofuse=True,
                        )
                        nop.engine = inst.engine
                        nop.sync_info = mybir.SyncInfo(on_wait=[w], on_update=[])
                        nc.register_instruction(nop)
                        out.append(nop)
                    inst.sync_info = mybir.SyncInfo(
                        on_wait=waits[-maxw:], on_update=list(si.on_update)
                    )
                out.append(inst)
            bb.instructions[:] = out

# ---------------------------------------------------------------------------

F16 = mybir.dt.float16

N_CORES = 8
B, C, H, W = 4, 64, 128, 128
BC = B * C                # 256 channels
CPC = BC // N_CORES       # 32 channels per core
NCH = 8                   # channels per group (one tile set)
NG = CPC // NCH           # groups per core
L = H * W
CH_FREE = 9 * 128         # 1152 elems per channel per partition
FREE = NCH * CH_FREE      # tile free width (9216)
OUT_CH = 9 * L            # 147456 elems per channel
G_ELEM = 128 * FREE       # elems per group buffer [128, FREE]


def _build_program():
    nc = bass.Bass(trn_type="TRN2")
    # host-prepared unfold operands in exact tile layout
    # [NG, 128, NCH, 1152] per input
    uk = nc.dram_tensor("uk", [NG * G_ELEM], F16, kind="ExternalInput")
    uq = nc.dram_tensor("uq", [NG * G_ELEM], F16, kind="ExternalInput")
    # outputs in the same layout (host permutes back)
    o1 = nc.dram_tensor("o1", [NG * G_ELEM], F16, kind="ExternalOutput")
    o2 = nc.dram_tensor("o2", [NG * G_ELEM], F16, kind="ExternalOutput")

    HF = FREE // 2  # free-dim half (4608): finer pipeline granularity

    def half(base, h):
        return [[FREE, 128], [1, HF]], base + h * HF

    def mul_aps(o, a, b, h):
        off = h * HF
        lin = [[FREE, 128], [9, HF // 9], [1, 9]]
        bc = [[FREE, 128], [9, HF // 9], [0, 9]]
        return (AP(o, off, lin), AP(a, off, lin), AP(b, off + 4, bc))

    flat = [[FREE, 128], [1, FREE]]

    with tile.TileContext(nc) as tc:
        with (
            tc.tile_pool(name="tin", bufs=2) as tin,
            tc.tile_pool(name="tout", bufs=2) as tout,
        ):
            for g in range(NG):
                tk = tin.tile([128, FREE], F16, tag="tk")
                tq = tin.tile([128, FREE], F16, tag="tq")
                o1t = tout.tile([128, FREE], F16, tag="o1t")
                o2t = tout.tile([128, FREE], F16, tag="o2t")
                tkh, tqh = tk[:].tensor, tq[:].tensor
                o1h, o2h = o1t[:].tensor, o2t[:].tensor
                # loads (halves) stream on the two HWDGE queues; stores
                # (full tiles) go to the SWDGE queue so a store waiting on
                # a multiply never head-of-line-blocks later loads.
                for h in (0, 1):
                    ap, off = half(g * G_ELEM, h)
                    nc.sync.dma_start(AP(tkh, h * HF, ap), AP(uk, off, ap))
                    nc.scalar.dma_start(AP(tqh, h * HF, ap), AP(uq, off, ap))
                    nc.vector.tensor_mul(*mul_aps(o1h, tkh, tqh, h))
                    nc.vector.tensor_mul(*mul_aps(o2h, tqh, tkh, h))
                for od, oth in ((o1, o1t), (o2, o2t)):
                    nc.gpsimd.dma_start(
                        AP(od, g * G_ELEM, flat),
                        AP(oth[:].tensor, 0, flat),
                    )
    _split_waits(nc)
    return nc


_NC_CACHE = []


def _get_nc():
    if not _NC_CACHE:
        _NC_CACHE.append(_build_program())
    return _NC_CACHE[0]


def _unfold_dev(x):
    """[B,C,H,W] f32 -> [BC, 9L] fp16 raw-flat unfold (nine contiguous
    padded-image windows), then regrouped to device tile order
    [BC/NCH groups of (128, NCH, 1152)] per core slice later."""
    xpad = np.pad(
        np.ascontiguousarray(x, dtype=np.float32).reshape(BC, H, W),
        ((0, 0), (1, 1), (1, 1)),
    ).astype(np.float16)
    u = np.empty((BC, 9, L), np.float16)
    for p in range(9):
        di, dj = divmod(p, 3)
        u[:, p, :] = xpad[:, di : di + H, dj : dj + W].reshape(BC, L)
    # [BC, 9L] -> [BC, 128, 1152] (m = 1152 r + f) -> core/group layout
    return u.reshape(BC, 128, CH_FREE)


def make_in_maps(key_map, query_map):
    ku = _unfold_dev(key_map)
    qu = _unfold_dev(query_map)
    maps = []
    for m in range(N_CORES):
        sl = slice(m * CPC, (m + 1) * CPC)
        # [CPC, 128, 1152] -> [NG, NCH, 128, 1152] -> [NG, 128, NCH, 1152]
        def dev(u):
            return np.ascontiguousarray(
                u[sl].reshape(NG, NCH, 128, CH_FREE).transpose(0, 2, 1, 3)
            ).reshape(-1)

        maps.append({"uk": dev(ku), "uq": dev(qu)})
    return maps


def assemble(results):
    # device layout [NG, 128, NCH, 1152] -> per-channel [CPC, 147456]
    def unshuffle(o):
        return (
            o.reshape(NG, 128, NCH, CH_FREE)
            .transpose(0, 2, 1, 3)
            .reshape(CPC, OUT_CH)
        )

    out1 = np.concatenate(
        [unshuffle(results[m]["o1"]) for m in range(N_CORES)], axis=0
    )
    out2 = np.concatenate(
        [unshuffle(results[m]["o2"]) for m in range(N_CORES)], axis=0
    )
    return (
        out1.astype(np.float32).reshape(B, C, L, 9),
        out2.astype(np.float32).reshape(B, C, L, 9),
    )


def kernel(key_map, query_map):
    nc = _get_nc()
    in_maps = make_in_maps(key_map, query_map)
    res = run_bass_kernel_spmd(nc, in_maps, core_ids=list(range(N_CORES)))
    return assemble(res.results)
